# revision 46
# baseline (speedup 1.0000x reference)
"""Segment-sum (scatter-add) kernel for Trainium2, SPMD over 8 NeuronCores.

Problem: out[n, :] = sum over edges e with X_node[e] == n of H[e, :]
  H [E=800000, 64] f32, X_node [E] int64, node_num N=50000 -> out [N, 64] f32.

Strategy
--------
Host-side sharding: edges are bucketed by destination node (each core owns a
contiguous node range chosen so per-core edge counts are ~equal).  Within a
core, nodes are greedily packed into "windows" of <= WN=32 consecutive nodes
whose edges fit in B blocks of 128 edges; every window is padded to exactly
B*128 edge slots so all 8 cores run one identical SPMD program.

Per 128-edge block the host precomputes a one-hot fp8 mask [128 edges x WN
nodes] (mask[e, j] = 1 iff X[e] == window_base + j) and splits H rows into a
3-way fp8(e4m3) cascade hi + mid/16 + lo/256 == H to ~2^-12 relative.  Pairs
of blocks are packed into 256-edge "super-blocks" in the DoubleRow interleave
layout: per partition [H3(e0) | H3(e1) | mask(e0) | mask(e1)], one fp8 tensor
(224 B/edge total - the kernel is HBM-bandwidth bound, so bytes are king).

Device kernel per core:
  PE:  psum[WN, 192] += mask.T @ [hi|mid|lo] as ONE DoubleRow fp8 matmul
       per super-block (K=256 virtual, ~60 ns per 128 edges warm);
       SB-super accumulation group per window; G_PS=8 windows per 4-bank
       PSUM tile, 2 tiles ping-pong (all 8 banks).
  ACT: tmp1 = psum_mid/16, tmp2 = psum_lo/256   (batched over G_PS windows)
  DVE: out = psum_hi + tmp1 + tmp2 -> [WN, D] f32
  DMA: sync ring streams packed chunks (ramped sizes so the first matmul
       starts after ~0.3 MB), gpsimd SWDGE ring does stores so they never
       queue ahead of loads.
Host gathers window rows into out[n0:n1, :] (pure layout, no arithmetic).
"""

import os

import numpy as np
import ml_dtypes

BF16 = np.dtype(ml_dtypes.bfloat16)
FP8 = np.dtype(ml_dtypes.float8_e4m3)

N_CORES = 8
P = 128
D = 64
WN = 32    # nodes per window (mask width)
G_PS = 8   # windows per PSUM bank / fold batch
CH = 16    # steady-state super-blocks (256 edges each) per DMA chunk


def _chunk_plan(S):
    """Chunk sizes (in super-blocks) ramp 4,4,8 then CH: the first matmul
    only waits for a small chunk instead of a full steady-state one."""
    sizes = []
    t = 0
    for s in (4, 4, 8):
        if t + s > S:
            break
        sizes.append(s)
        t += s
    while t < S:
        s = min(CH, S - t)
        sizes.append(s)
        t += s
    return sizes


# ----------------------------------------------------------------- planning
def _pack_windows(counts, n0, n1, B):
    """Greedily pack nodes [n0, n1) into windows of <=WN nodes whose total
    edge count fits in B*128 slots.  Returns list of (node_start, n_nodes)."""
    cap = B * P
    wins = []
    ws = n0
    acc = 0
    nn = 0
    for n in range(n0, n1):
        c = int(counts[n])
        if nn == WN or (acc + c > cap and nn > 0):
            wins.append((ws, nn))
            ws, acc, nn = n, 0, 0
        if c > cap:
            return None  # single node exceeds capacity; need bigger B
        acc += c
        nn += 1
    if nn > 0:
        wins.append((ws, nn))
    return wins


def _plan(X, N):
    """Choose core node ranges, B (blocks/window) and W (windows/core)."""
    E = X.shape[0]
    order = np.argsort(X, kind="stable")
    Xs = X[order]
    counts = np.bincount(X, minlength=N)
    cum = np.zeros(N + 1, dtype=np.int64)
    np.cumsum(counts, out=cum[1:])

    nb = [0]
    for c in range(1, N_CORES):
        nb.append(int(np.searchsorted(cum, round(E * c / N_CORES), side="left")))
    nb.append(N)

    b_lo = max(2, -(-int(counts.max()) // P))
    b_lo += b_lo % 2  # DoubleRow pairs blocks: B must be even
    best = None
    for B in range(b_lo, b_lo + 24, 2):
        wins_all = []
        ok = True
        for c in range(N_CORES):
            wins = _pack_windows(counts, nb[c], nb[c + 1], B)
            if wins is None:
                ok = False
                break
            wins_all.append(wins)
        if not ok:
            continue
        W = max(len(w) for w in wins_all)
        cost = W * B  # proportional to padded edges (dominant DMA)
        if best is None or cost < best[0]:
            best = (cost, B, W, wins_all)
    assert best is not None, "window packing failed"
    _, B, W, wins_all = best
    return order, Xs, cum, nb, B, W, wins_all


def _build_core_inputs(H32, order, Xs, cum, wins, B, W):
    """Build the padded, reordered device inputs for one core."""
    T = W * B
    idx = np.full(T * P, -1, dtype=np.int64)
    off = np.full(T * P, 255, dtype=np.int64)  # >= WN: all-zero mask row
    for w, (ns, nn) in enumerate(wins):
        e0 = int(cum[ns])
        e1 = int(cum[ns + nn])
        ec = e1 - e0
        s = w * B * P
        idx[s : s + ec] = order[e0:e1]
        off[s : s + ec] = Xs[e0:e1] - ns

    Hg = H32[np.maximum(idx, 0)]
    Hg[idx < 0] = 0.0
    # 3-way fp8 cascade: hi + mid/16 + lo/256 == H to ~2^-12 relative.
    # Power-of-2 scales keep every term in e4m3's normal range.
    hi = Hg.astype(FP8)
    r1 = Hg - hi.astype(np.float32)
    mid = (r1 * 16.0).astype(FP8)
    r2 = r1 - mid.astype(np.float32) / 16.0
    lo = (r2 * 256.0).astype(FP8)
    msk = (off[:, None] == np.arange(WN)[None, :]).astype(FP8)  # [T*P, WN]
    # DoubleRow super-block layout (2 blocks interleave on the k axis):
    # per partition row: [H3(e0) | H3(e1) | mask(e0) | mask(e1)]
    S = T // 2
    H3 = np.concatenate([hi, mid, lo], axis=1).reshape(S, 2, P, 3 * D)
    Mr = msk.reshape(S, 2, P, WN)
    pk = np.concatenate(
        [
            H3.transpose(0, 2, 1, 3).reshape(S, P, 6 * D),
            Mr.transpose(0, 2, 1, 3).reshape(S, P, 2 * WN),
        ],
        axis=2,
    )  # [S, P, 6D+2WN]
    pkt = np.ascontiguousarray(
        pk.transpose(1, 0, 2).reshape(P, S * (6 * D + 2 * WN))
    )
    return pkt


# ------------------------------------------------------------- device kernel
def _build_program(T, W, B):
    import concourse.bacc as bacc
    import concourse.tile as tile
    import concourse.mybir as mybir

    nc = bacc.Bacc("TRN2", target_bir_lowering=False, debug=False)
    fp8 = mybir.dt.float8e4
    f32 = mybir.dt.float32

    PKW = 6 * D + 2 * WN  # packed fp8 super-row: [H3(e0)|H3(e1)|mask(e0)|mask(e1)]
    SB = B // 2           # super-blocks per window
    S = T // 2
    # psum per-window region padded to 256 f32 (1 KiB) so no matmul's
    # [WN, 192] output straddles a 2 KiB PSUM bank boundary
    PSW = 256
    with tile.TileContext(nc) as tc:
        with tc.tile_pool(name="dram", bufs=1, space="DRAM") as dram:
            pkt = dram.tile([P, S * PKW], fp8, kind="ExternalInput")
            # [WN, W*D] layout: each fold's store is G_PS*D f32 = 1 KiB
            # contiguous per partition
            odev = dram.tile([WN, W * D], f32, kind="ExternalOutput")

            with tc.tile_pool(name="hbuf", bufs=10) as hpool, \
                 tc.tile_pool(name="psum", bufs=2, space="PSUM") as pspool, \
                 tc.tile_pool(name="tmpb", bufs=6) as tpool, \
                 tc.tile_pool(name="outb", bufs=6) as opool:

                chunk_starts = {}
                t_acc = 0
                for s in _chunk_plan(S):
                    chunk_starts[t_acc] = s
                    t_acc += s

                pk = None
                t0 = 0
                ps = None
                n_chunk = 0
                for w in range(W):
                    g = w % G_PS
                    if g == 0:
                        ps = pspool.tile([WN, G_PS, PSW], f32)
                    for b in range(SB):
                        t = w * SB + b
                        if t in chunk_starts:
                            ch = chunk_starts[t]
                            t0 = t
                            pk = hpool.tile([P, CH, PKW], fp8, tag="h")
                            n_chunk += 1
                            nc.sync.dma_start(
                                out=pk[:, :ch, :],
                                in_=pkt[:, t * PKW : (t + ch) * PKW].rearrange(
                                    "p (c d) -> p c d", c=ch
                                ),
                            )
                        rel = t - t0
                        nc.tensor.matmul(
                            out=ps[:, g, 0 : 3 * D],
                            lhsT=pk[:, rel, 6 * D : PKW].rearrange(
                                "p (k m) -> p k m", k=2
                            ),
                            rhs=pk[:, rel, 0 : 6 * D].rearrange(
                                "p (k n) -> p k n", k=2
                            ),
                            start=(b == 0),
                            stop=(b == SB - 1),
                            perf_mode=mybir.MatmulPerfMode.DoubleRow,
                        )
                    if g == G_PS - 1 or w == W - 1:
                        ng = g + 1
                        w0 = w - g
                        # PSUM reads split across ACT and DVE in parallel so
                        # the PSUM slot releases ~1.5us after group stop
                        # (release gates the PE's next accumulation tile)
                        tmp1 = tpool.tile([WN, G_PS, D], f32, tag="t1")
                        nc.scalar.mul(
                            out=tmp1[:, :ng, :],
                            in_=ps[:, :ng, D : 2 * D],
                            mul=1.0 / 16.0,
                        )
                        tmp2 = tpool.tile([WN, G_PS, D], f32, tag="t2")
                        nc.vector.tensor_scalar_mul(
                            tmp2[:, :ng, :],
                            ps[:, :ng, 2 * D : 3 * D],
                            1.0 / 256.0,
                        )
                        qt = opool.tile([WN, G_PS, D], f32, tag="q")
                        nc.vector.tensor_tensor(
                            out=qt[:, :ng, :],
                            in0=ps[:, :ng, 0:D],
                            in1=tmp1[:, :ng, :],
                            op=mybir.AluOpType.add,
                        )
                        ot = opool.tile([WN, G_PS, D], f32, tag="o")
                        nc.vector.tensor_tensor(
                            out=ot[:, :ng, :],
                            in0=qt[:, :ng, :],
                            in1=tmp2[:, :ng, :],
                            op=mybir.AluOpType.add,
                        )
                        # SWDGE ring: stores never block chunk loads
                        nc.gpsimd.dma_start(
                            out=odev[:, w0 * D : (w + 1) * D].rearrange(
                                "n (g f) -> n g f", g=ng
                            ),
                            in_=ot[:, :ng, :],
                        )
    nc.compile()
    return nc, pkt, odev


# --------------------------------------------------------------------- main
def kernel(H, X_node, node_num):
    from concourse import bass_utils

    H32 = np.asarray(H, dtype=np.float32)
    X = np.asarray(X_node).astype(np.int64)
    N = int(node_num)
    E = X.shape[0]
    assert H32.shape == (E, D)

    order, Xs, cum, nb, B, W, wins_all = _plan(X, N)
    T = W * B

    nc, pkt, odev = _build_program(T, W, B)
    in_maps = []
    for c in range(N_CORES):
        pkt_np = _build_core_inputs(H32, order, Xs, cum, wins_all[c], B, W)
        in_maps.append({pkt.name: pkt_np})

    trace = bool(int(os.environ.get("SEGSUM_TRACE", "0")))
    res = bass_utils.run_bass_kernel_spmd(
        nc, in_maps, core_ids=list(range(N_CORES)), trace=trace
    )
    if trace:
        kernel.last_exec_time_ns = res.exec_time_ns
        kernel.last_mean_exec_time_ns = res.mean_exec_time_ns
        kernel.last_trace = (
            res.instructions_and_trace[1] if res.instructions_and_trace else None
        )

    out = np.zeros((N, D), dtype=np.float32)
    for c in range(N_CORES):
        ot = res.results[c][odev.name].reshape(WN, W, D)  # [node_off, w, D]
        for w, (ns, nn) in enumerate(wins_all[c]):
            out[ns : ns + nn, :] = ot[:nn, w, :]
    return out
